# revision 15
# baseline (speedup 1.0000x reference)
"""Trainium2 Bass kernel for CausalHolographicQKV.

Math: unit_projection/bind/unbind are all pointwise in the Fourier domain
along D, so the FFTs fold into the Q/K/V projection weights (host-side DFT
of the weight matrices).  Real-input conjugate symmetry packs the spectrum
into exactly 1024 real channels: [Re F_0..511 | 0 | Im F_1..511], with the
Nyquist bin (Re F_512) handled by a tiny sidecar.  On device everything is:

  forward:  f = xT @ What (fp32r matmuls, PE)         [ch, s] layout
  bind:     p = (fk * fv) / (|fk||fv|)  (DVE/ACT elementwise)
  cumsum:   native tensor_tensor_scan along the free (s) axis
  unbind:   z = (s + offset) * conj(fq)/|fq|
  inverse:  out = zT @ G (fp32r matmuls; G = packed inverse DFT)

Sharding: core c = 2*b + h handles batch b, sequence half h (2048 rows).
The causal cumsum crosses the half boundary only through the total sum of
the first half -- a [128,16] per-pair AllReduce (masked so h0 contributes,
h1 consumes).
"""

import os
import sys

sys.path.insert(0, "/opt/trn_rl_repo")

import numpy as np

import concourse.bacc as bacc
import concourse.mybir as mybir
import concourse.tile as tile
from concourse.bass_utils import run_bass_kernel_spmd

F32 = mybir.dt.float32
F32R = mybir.dt.float32r
AO = mybir.AluOpType
ACT = mybir.ActivationFunctionType

B, S, D = 4, 4096, 1024
NCORES = 8
SC = S // 2          # rows per core
NB = SC // 512       # s-blocks per core
SBK = 512            # s-block size
KT = 8               # k (contraction) tiles of 128
CT = 8               # channel tiles (0..3 re, 4..7 im)
TINY = 1e-12


def _f32r_round(x):
    u = np.ascontiguousarray(np.asarray(x, np.float32)).view(np.uint32)
    r = ((u.astype(np.uint64) + 0x800) & 0xFFFFF000).astype(np.uint32)
    return r.view(np.float32)


def _pack_weight(W):
    # einsum('bsd,ed->bse', x, W) = x @ W.T ; spectrum of that along e.
    Wh = np.fft.rfft(np.asarray(W, np.float64).T, axis=1)  # [D, 513]
    Wt = np.zeros((D, D), np.float64)
    Wt[:, 0:512] = Wh.real[:, 0:512]
    Wt[:, 513:1024] = Wh.imag[:, 1:512]
    return Wt.astype(np.float32), Wh.real[:, 512].astype(np.float32)


def _pack_bias(b):
    bh = np.fft.rfft(np.asarray(b, np.float64))
    bt = np.zeros(D, np.float64)
    bt[0:512] = bh.real[0:512]
    bt[513:1024] = bh.imag[1:512]
    return bt.astype(np.float32), float(bh.real[512])


def _build_G():
    d = np.arange(D)
    G = np.zeros((D, D), np.float64)
    c = np.arange(512)[:, None]
    G[0:512] = (2.0 / D) * np.cos(2 * np.pi * c * d / D)
    G[0] *= 0.5
    G[512:1024] = -(2.0 / D) * np.sin(2 * np.pi * c * d / D)
    gny = (1.0 / D) * np.cos(np.pi * d)
    return G.astype(np.float32), gny.astype(np.float32)


def _build_nc(collective=True):
    nc = bacc.Bacc("TRN2", target_bir_lowering=False, debug=False,
                   num_devices=NCORES)

    xt = nc.dram_tensor("xt", [KT, 128, SC], F32R, kind="ExternalInput")
    wkv = nc.dram_tensor("wkv", [2 * KT, 128, D], F32R, kind="ExternalInput")
    wq = nc.dram_tensor("wq", [KT, 128, D], F32R, kind="ExternalInput")
    g = nc.dram_tensor("g", [CT, 128, D], F32R, kind="ExternalInput")
    wny = nc.dram_tensor("wny", [128, KT * 4], F32, kind="ExternalInput")  # cols: kt*4 + {k,v,q,0}
    aux1 = nc.dram_tensor("aux1", [1, 1156], F32R, kind="ExternalInput")
    u = nc.dram_tensor("u", [128, 128], F32, kind="ExternalInput")
    ident = nc.dram_tensor("ident", [128, 128], F32, kind="ExternalInput")
    bias = nc.dram_tensor("bias", [128, 24], F32, kind="ExternalInput")
    masks = nc.dram_tensor("masks", [128, 2], F32, kind="ExternalInput")
    outt = nc.dram_tensor("out", [SC, D], F32, kind="ExternalOutput")

    with tile.TileContext(nc) as tc:
        with (
            tc.tile_pool(name="persist", bufs=1) as pp,
            tc.tile_pool(name="gp", bufs=1) as gpp,
            tc.tile_pool(name="dramcc", bufs=1, space="DRAM") as dcc,
            tc.tile_pool(name="spill", bufs=1, space="DRAM") as spl,
        ):
            # prefetch the inverse-DFT weights during phase 1 so phase 2
            # doesn't stall on them
            g_t = gpp.tile([128, CT * D], F32R, tag="g")
            for ct in range(CT):
                nc.sync.dma_start(g_t[:, ct * D:(ct + 1) * D], g[ct, :, :])
            bias_t = pp.tile([128, 24], F32)
            nc.sync.dma_start(bias_t[:], bias[:, :])
            masks_t = pp.tile([128, 2], F32)
            nc.sync.dma_start(masks_t[:], masks[:, :])
            aux_t = pp.tile([1, 1156], F32R)
            nc.sync.dma_start(aux_t[:], aux1[:, :])
            u_t = pp.tile([128, 128], F32)
            nc.sync.dma_start(u_t[:], u[:, :])
            id_t = pp.tile([128, 128], F32)
            nc.sync.dma_start(id_t[:], ident[:, :])
            wny_t = pp.tile([128, KT * 4], F32)
            nc.sync.dma_start(wny_t[:], wny[:, :])

            sc_f = pp.tile([128, 32], F32)       # ny raw f (k|v channel-major)
            contrib = pp.tile([128, 16], F32)
            offs = pp.tile([128, 16], F32)
            zflat = pp.tile([1, SC // 128 * 128], F32R)  # [1, 2048] ny z row

            sp_tiles = [spl.tile([128, SC], F32, tag=f"sp{t}", name=f"sp{t}")
                        for t in range(CT)]

            # ---------------- phase 1: k,v forward + bind + scan ----------
            with (
                tc.tile_pool(name="wkvp", bufs=1) as wkvp,
                tc.tile_pool(name="xin", bufs=2) as xin,
                tc.tile_pool(name="shp", bufs=2) as shp,
                tc.tile_pool(name="tmp1", bufs=1) as tp1,
                tc.tile_pool(name="ps1", bufs=5, space="PSUM") as ps1,
                tc.tile_pool(name="psny", bufs=1, space="PSUM") as psny,
            ):
                # block-0 x first so the first matmul group starts early
                sbx0 = xin.tile([128, KT * SBK], F32R, tag="sbx", name="sbx_0")
                for kt in range(KT):
                    nc.sync.dma_start(sbx0[:, kt * SBK:(kt + 1) * SBK],
                                      xt[kt, :, 0:SBK])
                wkv_t = wkvp.tile([128, 2 * KT * D], F32R)
                for pk in range(2 * KT):
                    nc.sync.dma_start(wkv_t[:, pk * D:(pk + 1) * D],
                                      wkv[pk, :, :])

                def wslc(proj, kt, ctile):
                    base = proj * (KT * D) + kt * D + ctile * 128
                    return wkv_t[:, base:base + 128]

                prev_sh = [None] * CT
                for blk in range(NB):
                    s0 = blk * SBK
                    if blk == 0:
                        sbx = sbx0
                    else:
                        sbx = xin.tile([128, KT * SBK], F32R, tag="sbx",
                                       name=f"sbx_{blk}")
                        for kt in range(KT):
                            nc.sync.dma_start(sbx[:, kt * SBK:(kt + 1) * SBK],
                                              xt[kt, :, s0:s0 + SBK])

                    sh_blk = [shp.tile([128, SBK], F32, tag=f"sh{t}",
                                        name=f"sh{t}_{blk}")
                              for t in range(CT)]

                    for tp in range(4):
                        pk_re = ps1.tile([128, SBK], F32, tag="fwd")
                        pk_im = ps1.tile([128, SBK], F32, tag="fwd")
                        pv_re = ps1.tile([128, SBK], F32, tag="fwd")
                        pv_im = ps1.tile([128, SBK], F32, tag="fwd")
                        for psum_t, proj, ctile in (
                            (pk_re, 0, tp), (pk_im, 0, tp + 4),
                            (pv_re, 1, tp), (pv_im, 1, tp + 4),
                        ):
                            for kt in range(KT):
                                nc.tensor.matmul(
                                    psum_t[:], lhsT=wslc(proj, kt, ctile),
                                    rhs=sbx[:, kt * SBK:(kt + 1) * SBK],
                                    start=(kt == 0), stop=(kt == KT - 1))

                        b_kre = bias_t[:, tp:tp + 1]
                        b_kim = bias_t[:, 4 + tp:5 + tp]
                        b_vre = bias_t[:, 8 + tp:9 + tp]
                        b_vim = bias_t[:, 12 + tp:13 + tp]

                        sqa = tp1.tile([128, SBK], F32, tag="sqa")
                        nc.scalar.activation(sqa[:], pk_re[:], ACT.Square,
                                             bias=b_kre, scale=1.0)
                        sqb = tp1.tile([128, SBK], F32, tag="sqb")
                        nc.scalar.activation(sqb[:], pk_im[:], ACT.Square,
                                             bias=b_kim, scale=1.0)
                        sqc = tp1.tile([128, SBK], F32, tag="sqc")
                        nc.scalar.activation(sqc[:], pv_re[:], ACT.Square,
                                             bias=b_vre, scale=1.0)
                        sqd = tp1.tile([128, SBK], F32, tag="sqd")
                        nc.scalar.activation(sqd[:], pv_im[:], ACT.Square,
                                             bias=b_vim, scale=1.0)

                        kk = tp1.tile([128, SBK], F32, tag="kk")
                        nc.vector.scalar_tensor_tensor(
                            out=kk[:], in0=sqa[:], scalar=TINY, in1=sqb[:],
                            op0=AO.add, op1=AO.add)
                        vv = tp1.tile([128, SBK], F32, tag="vv")
                        nc.vector.scalar_tensor_tensor(
                            out=vv[:], in0=sqc[:], scalar=TINY, in1=sqd[:],
                            op0=AO.add, op1=AO.add)
                        mm = tp1.tile([128, SBK], F32, tag="sqa", name="mm_t")
                        nc.vector.tensor_mul(mm[:], kk[:], vv[:])
                        r2 = tp1.tile([128, SBK], F32, tag="sqb", name="r2_t")
                        nc.vector.reciprocal_approx_fast(out=r2[:], in_=mm[:])
                        rkv = tp1.tile([128, SBK], F32, tag="sqc", name="rkv_t")
                        nc.scalar.activation(rkv[:], r2[:], ACT.Sqrt)

                        kre = tp1.tile([128, SBK], F32, tag="kre")
                        nc.vector.scalar_tensor_tensor(
                            out=kre[:], in0=pk_re[:], scalar=b_kre, in1=rkv[:],
                            op0=AO.add, op1=AO.mult)
                        kim = tp1.tile([128, SBK], F32, tag="kim")
                        nc.vector.scalar_tensor_tensor(
                            out=kim[:], in0=pk_im[:], scalar=b_kim, in1=rkv[:],
                            op0=AO.add, op1=AO.mult)
                        vre = tp1.tile([128, SBK], F32, tag="vre")
                        nc.scalar.activation(vre[:], pv_re[:], ACT.Identity,
                                             bias=b_vre, scale=1.0)
                        vim = tp1.tile([128, SBK], F32, tag="vim")
                        nc.scalar.activation(vim[:], pv_im[:], ACT.Identity,
                                             bias=b_vim, scale=1.0)

                        t1 = tp1.tile([128, SBK], F32, tag="t1")
                        nc.vector.tensor_mul(t1[:], kre[:], vre[:])
                        t2 = tp1.tile([128, SBK], F32, tag="t2")
                        nc.vector.tensor_mul(t2[:], kim[:], vim[:])
                        t3 = tp1.tile([128, SBK], F32, tag="t3")
                        nc.vector.tensor_mul(t3[:], kre[:], vim[:])
                        t4 = tp1.tile([128, SBK], F32, tag="t4")
                        nc.vector.tensor_mul(t4[:], kim[:], vre[:])

                        init_re = 0.0 if blk == 0 else prev_sh[tp][:, SBK - 1:SBK]
                        nc.vector.tensor_tensor_scan(
                            out=sh_blk[tp][:], data0=t1[:], data1=t2[:],
                            initial=init_re, op0=AO.add, op1=AO.subtract)
                        init_im = 0.0 if blk == 0 else prev_sh[tp + 4][:, SBK - 1:SBK]
                        nc.vector.tensor_tensor_scan(
                            out=sh_blk[tp + 4][:], data0=t3[:], data1=t4[:],
                            initial=init_im, op0=AO.add, op1=AO.add)

                    # nyquist k,v forward for this block: psum [128 s, 2]
                    for j in range(SBK // 128):
                        pny = psny.tile([128, 2], F32, tag="ny", bufs=2)
                        for kt in range(KT):
                            nc.tensor.matmul(
                                pny[:],
                                lhsT=sbx[:, kt * SBK + j * 128:
                                         kt * SBK + (j + 1) * 128].bitcast(F32),
                                rhs=wny_t[:, kt * 4:kt * 4 + 2],
                                start=(kt == 0), stop=False)
                        nc.tensor.matmul(
                            pny[:], lhsT=aux_t[0:1, 0:128].bitcast(F32),
                            rhs=aux_t[0:1, 128:130].bitcast(F32),
                            start=False, stop=True)
                        idx = blk * (SBK // 128) + j
                        # channel-major: k -> col idx, v -> 16+idx
                        for ch in range(2):
                            nc.scalar.activation(
                                sc_f[:, ch * 16 + idx:ch * 16 + idx + 1],
                                pny[:, ch:ch + 1], ACT.Copy)

                    # spill scans of this block
                    for t in range(CT):
                        nc.sync.dma_start(sp_tiles[t][:, s0:s0 + SBK], sh_blk[t][:])
                    prev_sh = sh_blk

                # ---- nyquist normalize / bind / cumsum ----
                nsq = tp1.tile([128, 32], F32, tag="nsq")
                nc.vector.tensor_mul(nsq[:], sc_f[:], sc_f[:])
                nc.vector.tensor_scalar_add(nsq[:], nsq[:], TINY)
                nr2 = tp1.tile([128, 32], F32, tag="nr2")
                nc.vector.reciprocal_approx_fast(out=nr2[:], in_=nsq[:])
                nrr = tp1.tile([128, 32], F32, tag="nrr")
                nc.scalar.activation(nrr[:], nr2[:], ACT.Sqrt)
                nrm = tp1.tile([128, 32], F32, tag="nrm")
                nc.vector.tensor_mul(nrm[:], sc_f[:], nrr[:])
                sc_p = pp.tile([128, 16], F32)
                nc.vector.tensor_mul(sc_p[:], nrm[:, 0:16], nrm[:, 16:32])

                pcum = psny.tile([128, 16], F32, tag="nymisc", name="pcum")
                nc.tensor.matmul(pcum[:], lhsT=u_t[:], rhs=sc_p[:],
                                 start=True, stop=True)
                sc_cum = pp.tile([128, 16], F32)
                nc.vector.tensor_copy(sc_cum[:], pcum[:])
                # column totals into partition 0 (ones-column matmul; engines
                # cannot read from a nonzero base partition)
                ptot = psny.tile([1, 16], F32, tag="nymisc", name="ptot")
                nc.tensor.matmul(ptot[:], lhsT=u_t[:, 127:128], rhs=sc_p[:],
                                 start=True, stop=True)
                tot_row = pp.tile([1, 16], F32)
                nc.vector.tensor_copy(tot_row[:], ptot[:])
                cinc = pp.tile([1, 16], F32)
                nc.vector.tensor_tensor_scan(
                    out=cinc[:], data0=tot_row[:], data1=tot_row[:],
                    initial=0.0, op0=AO.add, op1=AO.bypass)
                coff = pp.tile([1, 16], F32)
                nc.vector.memset(coff[:], 0.0)
                nc.vector.tensor_copy(coff[0:1, 1:16], cinc[0:1, 0:15])
                coffb = pp.tile([128, 16], F32)
                nc.gpsimd.partition_broadcast(coffb[:], coff[:])
                sc_s = pp.tile([128, 16], F32)
                nc.vector.tensor_add(sc_s[:], sc_cum[:], coffb[:])

                # ---- totals + pairwise collective ----
                nc.vector.memset(contrib[:], 0.0)
                for t in range(CT):
                    nc.vector.tensor_copy(contrib[:, t:t + 1],
                                          prev_sh[t][:, SBK - 1:SBK])
                nc.vector.tensor_copy(contrib[0:1, 8:9], cinc[0:1, 15:16])
                contribm = pp.tile([128, 16], F32)
                nc.vector.tensor_scalar_mul(contribm[:], contrib[:],
                                            masks_t[:, 0:1])
                cc_in = dcc.tile([128, 16], F32)
                cc_out = dcc.tile([128, 16], F32)
                nc.sync.dma_start(cc_in[:], contribm[:])
                if collective:
                    nc.gpsimd.collective_compute(
                        "AllReduce", AO.add,
                        replica_groups=[[0, 1], [2, 3], [4, 5], [6, 7]],
                        ins=[cc_in[:].opt()], outs=[cc_out[:].opt()])
                else:
                    nc.sync.dma_start(cc_out[:], cc_in[:])
                offs_raw = pp.tile([128, 16], F32)
                nc.sync.dma_start(offs_raw[:], cc_out[:])
                nc.vector.tensor_scalar_mul(offs[:], offs_raw[:],
                                            masks_t[:, 1:2])

                # ny offset (z row built per-block in phase 2 with exact q)
                onyb = pp.tile([128, 1], F32)
                nc.gpsimd.partition_broadcast(onyb[:], offs[0:1, 8:9])
                sc_sg = pp.tile([128, 16], F32)
                nc.vector.tensor_scalar_add(sc_sg[:], sc_s[:], onyb[:])

            # ---------------- phase 2: q forward + unbind + inverse --------
            with (
                tc.tile_pool(name="wqp", bufs=1) as wqp,
                tc.tile_pool(name="xin2", bufs=2) as xin2,
                tc.tile_pool(name="shin", bufs=1) as shinp,
                tc.tile_pool(name="zp", bufs=1) as zp,
                tc.tile_pool(name="tmp2", bufs=1) as tp2,
                tc.tile_pool(name="ob", bufs=1) as obp,
                tc.tile_pool(name="ps2", bufs=3, space="PSUM") as ps2,
                tc.tile_pool(name="psi", bufs=3, space="PSUM") as psi,
            ):
                sbxf0 = xin2.tile([128, KT * SBK], F32R, tag="sbx2", bufs=2,
                                  name="sbxf_0")
                for kt in range(KT):
                    nc.sync.dma_start(sbxf0[:, kt * SBK:(kt + 1) * SBK],
                                      xt[kt, :, 0:SBK])
                wq_t = wqp.tile([128, KT * D], F32R, tag="wq")
                for pk in range(KT):
                    nc.sync.dma_start(wq_t[:, pk * D:(pk + 1) * D], wq[pk, :, :])

                for blk in range(NB):
                    s0 = blk * SBK
                    if blk == 0:
                        sbx2 = sbxf0
                    else:
                        sbx2 = xin2.tile([128, KT * SBK], F32R, tag="sbx2",
                                         bufs=2, name=f"sbxf_{blk}")
                        for kt in range(KT):
                            nc.sync.dma_start(sbx2[:, kt * SBK:(kt + 1) * SBK],
                                              xt[kt, :, s0:s0 + SBK])
                    shin = [shinp.tile([128, SBK], F32, tag=f"shi{t}",
                                        name=f"shi{t}_{blk}")
                            for t in range(CT)]
                    for t in range(CT):
                        nc.sync.dma_start(shin[t][:], sp_tiles[t][:, s0:s0 + SBK])
                    zts = [zp.tile([128, SBK], F32R, tag=f"z{t}",
                                    name=f"z{t}_{blk}")
                           for t in range(CT)]

                    # exact nyquist-q for this block -> z row entries
                    nyq = tp2.tile([128, SBK // 128], F32, tag="nyq",
                                   name=f"nyq_{blk}")
                    for j in range(SBK // 128):
                        pnyq = psi.tile([128, 2], F32, tag="nyq", bufs=2)
                        for kt in range(KT):
                            nc.tensor.matmul(
                                pnyq[:],
                                lhsT=sbx2[:, kt * SBK + j * 128:
                                          kt * SBK + (j + 1) * 128].bitcast(F32),
                                rhs=wny_t[:, kt * 4 + 2:kt * 4 + 4],
                                start=(kt == 0), stop=False)
                        nc.tensor.matmul(
                            pnyq[:], lhsT=aux_t[0:1, 0:128].bitcast(F32),
                            rhs=aux_t[0:1, 130:132].bitcast(F32),
                            start=False, stop=True)
                        nc.scalar.activation(nyq[:, j:j + 1], pnyq[:, 0:1], ACT.Copy)
                    nqs = tp2.tile([128, SBK // 128], F32, tag="nqs")
                    nc.vector.tensor_mul(nqs[:], nyq[:], nyq[:])
                    nc.vector.tensor_scalar_add(nqs[:], nqs[:], TINY)
                    nqr = tp2.tile([128, SBK // 128], F32, tag="nqr")
                    nc.vector.reciprocal_approx_fast(out=nqr[:], in_=nqs[:])
                    nc.scalar.activation(nqr[:], nqr[:], ACT.Sqrt)
                    znyb = tp2.tile([128, 16], F32, tag="znyb",
                                    name=f"znyb_{blk}")
                    nc.vector.memset(znyb[:], 0.0)
                    nc.vector.tensor_mul(znyb[:, 0:SBK // 128], nyq[:], nqr[:])
                    nc.vector.tensor_mul(
                        znyb[:, 0:SBK // 128], znyb[:, 0:SBK // 128],
                        sc_sg[:, blk * (SBK // 128):(blk + 1) * (SBK // 128)])
                    pzt = psi.tile([16, 128], F32, tag="nyq", bufs=2,
                                   name=f"pzt_{blk}")
                    nc.tensor.transpose(pzt[:], znyb[:], id_t[:])
                    zt4 = tp2.tile([16, 128], F32R, tag="zt4",
                                   name=f"zt4_{blk}")
                    nc.vector.tensor_copy(zt4[:], pzt[:])
                    for r in range(SBK // 128):
                        row = blk * (SBK // 128) + r
                        nc.sync.dma_start(zflat[0:1, row * 128:(row + 1) * 128],
                                          zt4[r:r + 1, :])

                    for tp in range(4):
                        pq_re = ps2.tile([128, SBK], F32, tag="q")
                        pq_im = ps2.tile([128, SBK], F32, tag="q")
                        for psum_t, ctile in ((pq_re, tp), (pq_im, tp + 4)):
                            for kt in range(KT):
                                nc.tensor.matmul(
                                    psum_t[:],
                                    lhsT=wq_t[:, kt * D + ctile * 128:
                                              kt * D + ctile * 128 + 128],
                                    rhs=sbx2[:, kt * SBK:(kt + 1) * SBK],
                                    start=(kt == 0),
                                    stop=(kt == KT - 1))

                        b_qre = bias_t[:, 16 + tp:17 + tp]
                        b_qim = bias_t[:, 20 + tp:21 + tp]
                        sqa2 = tp2.tile([128, SBK], F32, tag="sqa2")
                        nc.scalar.activation(sqa2[:], pq_re[:], ACT.Square,
                                             bias=b_qre, scale=1.0)
                        sqb2 = tp2.tile([128, SBK], F32, tag="sqb2")
                        nc.scalar.activation(sqb2[:], pq_im[:], ACT.Square,
                                             bias=b_qim, scale=1.0)
                        qq = tp2.tile([128, SBK], F32, tag="qq")
                        nc.vector.scalar_tensor_tensor(
                            out=qq[:], in0=sqa2[:], scalar=TINY, in1=sqb2[:],
                            op0=AO.add, op1=AO.add)
                        r2q = tp2.tile([128, SBK], F32, tag="r2q")
                        nc.vector.reciprocal_approx_fast(out=r2q[:], in_=qq[:])
                        rq = tp2.tile([128, SBK], F32, tag="rq")
                        nc.scalar.activation(rq[:], r2q[:], ACT.Sqrt)
                        qre = tp2.tile([128, SBK], F32, tag="sqa2", name="qre_t")
                        nc.vector.scalar_tensor_tensor(
                            out=qre[:], in0=pq_re[:], scalar=b_qre, in1=rq[:],
                            op0=AO.add, op1=AO.mult)
                        qim = tp2.tile([128, SBK], F32, tag="sqb2", name="qim_t")
                        nc.vector.scalar_tensor_tensor(
                            out=qim[:], in0=pq_im[:], scalar=b_qim, in1=rq[:],
                            op0=AO.add, op1=AO.mult)

                        o_re = offs[:, tp:tp + 1]
                        o_im = offs[:, 4 + tp:5 + tp]
                        t5 = tp2.tile([128, SBK], F32, tag="t5")
                        nc.vector.scalar_tensor_tensor(
                            out=t5[:], in0=shin[tp][:], scalar=o_re,
                            in1=qre[:], op0=AO.add, op1=AO.mult)
                        t6 = tp2.tile([128, SBK], F32, tag="t6")
                        nc.vector.scalar_tensor_tensor(
                            out=t6[:], in0=shin[tp + 4][:], scalar=o_im,
                            in1=qim[:], op0=AO.add, op1=AO.mult)
                        nc.vector.tensor_add(zts[tp][:], t5[:], t6[:])
                        t7 = tp2.tile([128, SBK], F32, tag="t5", name="t7_t")
                        nc.vector.scalar_tensor_tensor(
                            out=t7[:], in0=shin[tp + 4][:], scalar=o_im,
                            in1=qre[:], op0=AO.add, op1=AO.mult)
                        t8 = tp2.tile([128, SBK], F32, tag="t6", name="t8_t")
                        nc.vector.scalar_tensor_tensor(
                            out=t8[:], in0=shin[tp][:], scalar=o_re,
                            in1=qim[:], op0=AO.add, op1=AO.mult)
                        nc.vector.tensor_sub(zts[tp + 4][:], t7[:], t8[:])

                    for ss in range(SBK // 128):
                        row = blk * (SBK // 128) + ss
                        for dh in range(2):
                            po = psi.tile([128, 512], F32, tag="inv")
                            for ct in range(CT):
                                nc.tensor.matmul(
                                    po[:],
                                    lhsT=zts[ct][:, ss * 128:(ss + 1) * 128],
                                    rhs=g_t[:, ct * D + dh * 512:
                                            ct * D + dh * 512 + 512],
                                    start=(ct == 0), stop=False)
                            nc.tensor.matmul(
                                po[:],
                                lhsT=zflat[0:1, row * 128:(row + 1) * 128],
                                rhs=aux_t[0:1, 132 + dh * 512:132 + dh * 512 + 512],
                                start=False, stop=True)
                            ob = obp.tile([128, 512], F32, tag="ob", bufs=3)
                            nc.scalar.activation(ob[:], po[:], ACT.Copy)
                            nc.sync.dma_start(
                                outt[s0 + ss * 128:s0 + (ss + 1) * 128,
                                     dh * 512:(dh + 1) * 512], ob[:])

    nc.compile()
    return nc


_NC_CACHE = None


def _prep_in_maps(x, Wq, bq, Wk, bk, Wv, bv):
    x = np.asarray(x, np.float32)
    Wtq, wnyq = _pack_weight(Wq)
    btq, bnyq = _pack_bias(bq)
    Wtk, wnyk = _pack_weight(Wk)
    btk, bnyk = _pack_bias(bk)
    Wtv, wnyv = _pack_weight(Wv)
    btv, bnyv = _pack_bias(bv)
    G, gny = _build_G()

    def ktile(W):  # [D, D] -> [KT, 128, D]
        return _f32r_round(np.ascontiguousarray(W.reshape(KT, 128, D)))

    wkv_h = np.concatenate([ktile(Wtk), ktile(Wtv)])    # [2*KT, 128, D]
    wq_h = ktile(Wtq)
    g_h = _f32r_round(np.ascontiguousarray(G.reshape(CT, 128, D)))
    wny_kpc = np.stack(
        [wnyk, wnyv, wnyq, np.zeros_like(wnyk)], axis=1).reshape(KT, 128, 4)
    wny_h = np.ascontiguousarray(
        wny_kpc.transpose(1, 0, 2).reshape(128, KT * 4))
    aux_h = np.zeros((1, 1156), np.float32)
    aux_h[0, 0:128] = 1.0
    aux_h[0, 128:131] = [bnyk, bnyv, bnyq]
    aux_h[0, 132:1156] = gny
    aux_h = _f32r_round(aux_h)
    u_h = np.triu(np.ones((128, 128), np.float32))
    id_h = np.eye(128, dtype=np.float32)
    bias_h = np.zeros((128, 24), np.float32)
    for pi, bt in enumerate((btk, btv, btq)):
        bias_h[:, pi * 8:(pi + 1) * 8] = bt.reshape(8, 128).T

    xr = _f32r_round(x)
    in_maps = []
    for c in range(NCORES):
        b, h = c // 2, c % 2
        xs = np.ascontiguousarray(
            xr[b, h * SC:(h + 1) * SC, :].T.reshape(KT, 128, SC))
        m = np.zeros((128, 2), np.float32)
        m[:, 0] = 1.0 if h == 0 else 0.0
        m[:, 1] = 0.0 if h == 0 else 1.0
        in_maps.append(dict(
            xt=xs, wkv=wkv_h, wq=wq_h, g=g_h, wny=wny_h, aux1=aux_h,
            u=u_h, ident=id_h, bias=bias_h, masks=m))
    return in_maps


def kernel(x, Wq, bq, Wk, bk, Wv, bv):
    global _NC_CACHE

    in_maps = _prep_in_maps(x, Wq, bq, Wk, bk, Wv, bv)

    if _NC_CACHE is None:
        _NC_CACHE = _build_nc()
    nc = _NC_CACHE

    trace = bool(int(os.environ.get("KERNEL_TRACE", "0")))
    res = None
    if trace:
        try:
            res = run_bass_kernel_spmd(nc, in_maps, core_ids=list(range(NCORES)),
                                       trace=True)
        except Exception as e:  # ntff hook missing on older axon clients
            print(f"trace unavailable ({e}); rerunning without trace", flush=True)
            res = None
    if res is None:
        res = run_bass_kernel_spmd(nc, in_maps, core_ids=list(range(NCORES)))
    if res.exec_time_ns is not None:
        print(f"HW exec time: {res.exec_time_ns} ns", flush=True)
        kernel.last_exec_time_ns = res.exec_time_ns

    out = np.zeros((B, S, D), np.float32)
    for c in range(NCORES):
        b, h = c // 2, c % 2
        out[b, h * SC:(h + 1) * SC, :] = res.results[c]["out"]
    return out



# revision 18
# speedup vs baseline: 1.0347x; 1.0347x over previous
"""Trainium2 Bass kernel for CausalHolographicQKV.

Math: unit_projection/bind/unbind are all pointwise in the Fourier domain
along D, so the FFTs fold into the Q/K/V projection weights (host-side DFT
of the weight matrices).  Real-input conjugate symmetry packs the spectrum
into exactly 1024 real channels: [Re F_0..511 | 0 | Im F_1..511], with the
Nyquist bin (Re F_512) handled by a tiny sidecar.  On device everything is:

  forward:  f = xT @ What (fp32r matmuls, PE)         [ch, s] layout
  bind:     p = (fk * fv) / (|fk||fv|)  (DVE/ACT elementwise)
  cumsum:   native tensor_tensor_scan along the free (s) axis
  unbind:   z = (s + offset) * conj(fq)/|fq|
  inverse:  out = zT @ G (fp32r matmuls; G = packed inverse DFT)

Sharding: core c = 2*b + h handles batch b, sequence half h (2048 rows).
The causal cumsum crosses the half boundary only through the total sum of
the first half -- a [128,16] per-pair AllReduce (masked so h0 contributes,
h1 consumes).
"""

import os
import sys

sys.path.insert(0, "/opt/trn_rl_repo")

import numpy as np

import concourse.bacc as bacc
import concourse.mybir as mybir
import concourse.tile as tile
from concourse.bass_utils import run_bass_kernel_spmd

F32 = mybir.dt.float32
F32R = mybir.dt.float32r
AO = mybir.AluOpType
ACT = mybir.ActivationFunctionType

B, S, D = 4, 4096, 1024
NCORES = 8
SC = S // 2          # rows per core
NB = SC // 512       # s-blocks per core
SBK = 512            # s-block size
KT = 8               # k (contraction) tiles of 128
CT = 8               # channel tiles (0..3 re, 4..7 im)
TINY = 1e-12


def _f32r_round(x):
    u = np.ascontiguousarray(np.asarray(x, np.float32)).view(np.uint32)
    r = ((u.astype(np.uint64) + 0x800) & 0xFFFFF000).astype(np.uint32)
    return r.view(np.float32)


def _pack_weight(W):
    # einsum('bsd,ed->bse', x, W) = x @ W.T ; spectrum of that along e.
    Wh = np.fft.rfft(np.asarray(W, np.float64).T, axis=1)  # [D, 513]
    Wt = np.zeros((D, D), np.float64)
    Wt[:, 0:512] = Wh.real[:, 0:512]
    Wt[:, 513:1024] = Wh.imag[:, 1:512]
    return Wt.astype(np.float32), Wh.real[:, 512].astype(np.float32)


def _pack_bias(b):
    bh = np.fft.rfft(np.asarray(b, np.float64))
    bt = np.zeros(D, np.float64)
    bt[0:512] = bh.real[0:512]
    bt[513:1024] = bh.imag[1:512]
    return bt.astype(np.float32), float(bh.real[512])


def _build_G():
    d = np.arange(D)
    G = np.zeros((D, D), np.float64)
    c = np.arange(512)[:, None]
    G[0:512] = (2.0 / D) * np.cos(2 * np.pi * c * d / D)
    G[0] *= 0.5
    G[512:1024] = -(2.0 / D) * np.sin(2 * np.pi * c * d / D)
    gny = (1.0 / D) * np.cos(np.pi * d)
    return G.astype(np.float32), gny.astype(np.float32)


def _build_nc(collective=True):
    nc = bacc.Bacc("TRN2", target_bir_lowering=False, debug=False,
                   num_devices=NCORES)

    xt = nc.dram_tensor("xt", [KT, 128, SC], F32R, kind="ExternalInput")
    wkv = nc.dram_tensor("wkv", [2 * KT, 128, D], F32R, kind="ExternalInput")
    wq = nc.dram_tensor("wq", [KT, 128, D], F32R, kind="ExternalInput")
    g = nc.dram_tensor("g", [CT, 128, D], F32R, kind="ExternalInput")
    wny = nc.dram_tensor("wny", [128, KT * 4], F32, kind="ExternalInput")  # cols: kt*4 + {k,v,q,0}
    aux1 = nc.dram_tensor("aux1", [1, 1156], F32R, kind="ExternalInput")
    u = nc.dram_tensor("u", [128, 128], F32, kind="ExternalInput")
    ident = nc.dram_tensor("ident", [128, 128], F32, kind="ExternalInput")
    bias = nc.dram_tensor("bias", [128, 24], F32, kind="ExternalInput")
    masks = nc.dram_tensor("masks", [128, 2], F32, kind="ExternalInput")
    outt = nc.dram_tensor("out", [SC, D], F32, kind="ExternalOutput")

    with tile.TileContext(nc) as tc:
        with (
            tc.tile_pool(name="persist", bufs=1) as pp,
            tc.tile_pool(name="gp", bufs=1) as gpp,
            tc.tile_pool(name="dramcc", bufs=1, space="DRAM") as dcc,
            tc.tile_pool(name="spill", bufs=1, space="DRAM") as spl,
        ):
            g_t = gpp.tile([128, CT * D], F32R, tag="g")
            bias_t = pp.tile([128, 24], F32)
            nc.sync.dma_start(bias_t[:], bias[:, :])
            masks_t = pp.tile([128, 2], F32)
            nc.sync.dma_start(masks_t[:], masks[:, :])
            aux_t = pp.tile([1, 1156], F32R)
            nc.sync.dma_start(aux_t[:], aux1[:, :])
            u_t = pp.tile([128, 128], F32)
            nc.sync.dma_start(u_t[:], u[:, :])
            id_t = pp.tile([128, 128], F32)
            nc.sync.dma_start(id_t[:], ident[:, :])
            wny_t = pp.tile([128, KT * 4], F32)
            nc.sync.dma_start(wny_t[:], wny[:, :])

            sc_f = pp.tile([128, 32], F32)       # ny raw f (k|v channel-major)
            contrib = pp.tile([128, 16], F32)
            offs = pp.tile([128, 16], F32)
            zflat = pp.tile([1, SC // 128 * 128], F32R)  # [1, 2048] ny z row

            sp_tiles = [spl.tile([128, SC], F32, tag=f"sp{t}", name=f"sp{t}")
                        for t in range(CT)]

            # ---------------- phase 1: k,v forward + bind + scan ----------
            with (
                tc.tile_pool(name="wkvp", bufs=1) as wkvp,
                tc.tile_pool(name="xin", bufs=2) as xin,
                tc.tile_pool(name="shp", bufs=2) as shp,
                tc.tile_pool(name="tmp1", bufs=1) as tp1,
                tc.tile_pool(name="ps1", bufs=6, space="PSUM") as ps1,
                tc.tile_pool(name="psny", bufs=1, space="PSUM") as psny,
            ):
                # block-0 x first so the first matmul group starts early
                sbx0 = xin.tile([128, KT * SBK], F32R, tag="sbx", name="sbx_0")
                for kt in range(KT):
                    nc.sync.dma_start(sbx0[:, kt * SBK:(kt + 1) * SBK],
                                      xt[kt, :, 0:SBK])
                wkv_t = wkvp.tile([128, 2 * KT * D], F32R)
                for pk in range(2 * KT):
                    nc.sync.dma_start(wkv_t[:, pk * D:(pk + 1) * D],
                                      wkv[pk, :, :])
                # prefetch the inverse-DFT weights during phase 1 (after the
                # startup-critical wkv/x loads) so phase 2 doesn't stall
                for ct in range(CT):
                    nc.sync.dma_start(g_t[:, ct * D:(ct + 1) * D], g[ct, :, :])

                def wslc(proj, kt, ctile):
                    base = proj * (KT * D) + kt * D + ctile * 128
                    return wkv_t[:, base:base + 128]

                prev_sh = [None] * CT
                for blk in range(NB):
                    s0 = blk * SBK
                    if blk == 0:
                        sbx = sbx0
                    else:
                        sbx = xin.tile([128, KT * SBK], F32R, tag="sbx",
                                       name=f"sbx_{blk}")
                        for kt in range(KT):
                            nc.sync.dma_start(sbx[:, kt * SBK:(kt + 1) * SBK],
                                              xt[kt, :, s0:s0 + SBK])

                    sh_blk = [shp.tile([128, SBK], F32, tag=f"sh{t}",
                                        name=f"sh{t}_{blk}")
                              for t in range(CT)]

                    for tp in range(4):
                        pk_re = ps1.tile([128, SBK], F32, tag="fwd")
                        pk_im = ps1.tile([128, SBK], F32, tag="fwd")
                        pv_re = ps1.tile([128, SBK], F32, tag="fwd")
                        pv_im = ps1.tile([128, SBK], F32, tag="fwd")
                        for psum_t, proj, ctile in (
                            (pk_re, 0, tp), (pk_im, 0, tp + 4),
                            (pv_re, 1, tp), (pv_im, 1, tp + 4),
                        ):
                            for kt in range(KT):
                                nc.tensor.matmul(
                                    psum_t[:], lhsT=wslc(proj, kt, ctile),
                                    rhs=sbx[:, kt * SBK:(kt + 1) * SBK],
                                    start=(kt == 0), stop=(kt == KT - 1))

                        b_kre = bias_t[:, tp:tp + 1]
                        b_kim = bias_t[:, 4 + tp:5 + tp]
                        b_vre = bias_t[:, 8 + tp:9 + tp]
                        b_vim = bias_t[:, 12 + tp:13 + tp]

                        sqa = tp1.tile([128, SBK], F32, tag="sqa")
                        nc.scalar.activation(sqa[:], pk_re[:], ACT.Square,
                                             bias=b_kre, scale=1.0)
                        sqb = tp1.tile([128, SBK], F32, tag="sqb")
                        nc.scalar.activation(sqb[:], pk_im[:], ACT.Square,
                                             bias=b_kim, scale=1.0)
                        sqc = tp1.tile([128, SBK], F32, tag="sqc")
                        nc.scalar.activation(sqc[:], pv_re[:], ACT.Square,
                                             bias=b_vre, scale=1.0)
                        sqd = tp1.tile([128, SBK], F32, tag="sqd")
                        nc.scalar.activation(sqd[:], pv_im[:], ACT.Square,
                                             bias=b_vim, scale=1.0)

                        kk = tp1.tile([128, SBK], F32, tag="kk")
                        nc.vector.scalar_tensor_tensor(
                            out=kk[:], in0=sqa[:], scalar=TINY, in1=sqb[:],
                            op0=AO.add, op1=AO.add)
                        vv = tp1.tile([128, SBK], F32, tag="vv")
                        nc.vector.scalar_tensor_tensor(
                            out=vv[:], in0=sqc[:], scalar=TINY, in1=sqd[:],
                            op0=AO.add, op1=AO.add)
                        mm = tp1.tile([128, SBK], F32, tag="sqa", name="mm_t")
                        nc.vector.tensor_mul(mm[:], kk[:], vv[:])
                        r2 = tp1.tile([128, SBK], F32, tag="sqb", name="r2_t")
                        nc.vector.reciprocal_approx_fast(out=r2[:], in_=mm[:])
                        rkv = tp1.tile([128, SBK], F32, tag="sqc", name="rkv_t")
                        nc.scalar.activation(rkv[:], r2[:], ACT.Sqrt)

                        kre = tp1.tile([128, SBK], F32, tag="kre")
                        nc.vector.scalar_tensor_tensor(
                            out=kre[:], in0=pk_re[:], scalar=b_kre, in1=rkv[:],
                            op0=AO.add, op1=AO.mult)
                        kim = tp1.tile([128, SBK], F32, tag="kim")
                        nc.vector.scalar_tensor_tensor(
                            out=kim[:], in0=pk_im[:], scalar=b_kim, in1=rkv[:],
                            op0=AO.add, op1=AO.mult)
                        vre = tp1.tile([128, SBK], F32, tag="vre")
                        nc.scalar.activation(vre[:], pv_re[:], ACT.Identity,
                                             bias=b_vre, scale=1.0)
                        vim = tp1.tile([128, SBK], F32, tag="vim")
                        nc.scalar.activation(vim[:], pv_im[:], ACT.Identity,
                                             bias=b_vim, scale=1.0)

                        t1 = tp1.tile([128, SBK], F32, tag="t1")
                        nc.vector.tensor_mul(t1[:], kre[:], vre[:])
                        t2 = tp1.tile([128, SBK], F32, tag="t2")
                        nc.vector.tensor_mul(t2[:], kim[:], vim[:])
                        t3 = tp1.tile([128, SBK], F32, tag="t3")
                        nc.vector.tensor_mul(t3[:], kre[:], vim[:])
                        t4 = tp1.tile([128, SBK], F32, tag="t4")
                        nc.vector.tensor_mul(t4[:], kim[:], vre[:])

                        init_re = 0.0 if blk == 0 else prev_sh[tp][:, SBK - 1:SBK]
                        nc.vector.tensor_tensor_scan(
                            out=sh_blk[tp][:], data0=t1[:], data1=t2[:],
                            initial=init_re, op0=AO.add, op1=AO.subtract)
                        init_im = 0.0 if blk == 0 else prev_sh[tp + 4][:, SBK - 1:SBK]
                        nc.vector.tensor_tensor_scan(
                            out=sh_blk[tp + 4][:], data0=t3[:], data1=t4[:],
                            initial=init_im, op0=AO.add, op1=AO.add)

                    # nyquist k,v forward for this block: psum [128 s, 2]
                    for j in range(SBK // 128):
                        pny = psny.tile([128, 2], F32, tag="ny", bufs=2)
                        for kt in range(KT):
                            nc.tensor.matmul(
                                pny[:],
                                lhsT=sbx[:, kt * SBK + j * 128:
                                         kt * SBK + (j + 1) * 128].bitcast(F32),
                                rhs=wny_t[:, kt * 4:kt * 4 + 2],
                                start=(kt == 0), stop=False)
                        nc.tensor.matmul(
                            pny[:], lhsT=aux_t[0:1, 0:128].bitcast(F32),
                            rhs=aux_t[0:1, 128:130].bitcast(F32),
                            start=False, stop=True)
                        idx = blk * (SBK // 128) + j
                        # channel-major: k -> col idx, v -> 16+idx
                        for ch in range(2):
                            nc.scalar.activation(
                                sc_f[:, ch * 16 + idx:ch * 16 + idx + 1],
                                pny[:, ch:ch + 1], ACT.Copy)

                    # spill scans of this block
                    for t in range(CT):
                        nc.sync.dma_start(sp_tiles[t][:, s0:s0 + SBK], sh_blk[t][:])
                    prev_sh = sh_blk

                # ---- nyquist normalize / bind / cumsum ----
                nsq = tp1.tile([128, 32], F32, tag="nsq")
                nc.vector.tensor_mul(nsq[:], sc_f[:], sc_f[:])
                nc.vector.tensor_scalar_add(nsq[:], nsq[:], TINY)
                nr2 = tp1.tile([128, 32], F32, tag="nr2")
                nc.vector.reciprocal_approx_fast(out=nr2[:], in_=nsq[:])
                nrr = tp1.tile([128, 32], F32, tag="nrr")
                nc.scalar.activation(nrr[:], nr2[:], ACT.Sqrt)
                nrm = tp1.tile([128, 32], F32, tag="nrm")
                nc.vector.tensor_mul(nrm[:], sc_f[:], nrr[:])
                sc_p = pp.tile([128, 16], F32)
                nc.vector.tensor_mul(sc_p[:], nrm[:, 0:16], nrm[:, 16:32])

                pcum = ps1.tile([128, 16], F32, tag="fwd", name="pcum")
                nc.tensor.matmul(pcum[:], lhsT=u_t[:], rhs=sc_p[:],
                                 start=True, stop=True)
                sc_cum = pp.tile([128, 16], F32)
                nc.vector.tensor_copy(sc_cum[:], pcum[:])
                # column totals into partition 0 (ones-column matmul; engines
                # cannot read from a nonzero base partition)
                ptot = ps1.tile([1, 16], F32, tag="fwd", name="ptot")
                nc.tensor.matmul(ptot[:], lhsT=u_t[:, 127:128], rhs=sc_p[:],
                                 start=True, stop=True)
                tot_row = pp.tile([1, 16], F32)
                nc.vector.tensor_copy(tot_row[:], ptot[:])
                cinc = pp.tile([1, 16], F32)
                nc.vector.tensor_tensor_scan(
                    out=cinc[:], data0=tot_row[:], data1=tot_row[:],
                    initial=0.0, op0=AO.add, op1=AO.bypass)
                coff = pp.tile([1, 16], F32)
                nc.vector.memset(coff[:], 0.0)
                nc.vector.tensor_copy(coff[0:1, 1:16], cinc[0:1, 0:15])
                coffb = pp.tile([128, 16], F32)
                nc.gpsimd.partition_broadcast(coffb[:], coff[:])
                sc_s = pp.tile([128, 16], F32)
                nc.vector.tensor_add(sc_s[:], sc_cum[:], coffb[:])

                # ---- totals + pairwise collective ----
                nc.vector.memset(contrib[:], 0.0)
                for t in range(CT):
                    nc.vector.tensor_copy(contrib[:, t:t + 1],
                                          prev_sh[t][:, SBK - 1:SBK])
                nc.vector.tensor_copy(contrib[0:1, 8:9], cinc[0:1, 15:16])
                contribm = pp.tile([128, 16], F32)
                nc.vector.tensor_scalar_mul(contribm[:], contrib[:],
                                            masks_t[:, 0:1])
                cc_in = dcc.tile([128, 16], F32)
                cc_out = dcc.tile([128, 16], F32)
                nc.sync.dma_start(cc_in[:], contribm[:])
                if collective:
                    nc.gpsimd.collective_compute(
                        "AllReduce", AO.add,
                        replica_groups=[[0, 1], [2, 3], [4, 5], [6, 7]],
                        ins=[cc_in[:].opt()], outs=[cc_out[:].opt()])
                else:
                    nc.sync.dma_start(cc_out[:], cc_in[:])
                offs_raw = pp.tile([128, 16], F32)
                nc.sync.dma_start(offs_raw[:], cc_out[:])
                nc.vector.tensor_scalar_mul(offs[:], offs_raw[:],
                                            masks_t[:, 1:2])

                # ny offset (z row built per-block in phase 2 with exact q)
                onyb = pp.tile([128, 1], F32)
                nc.gpsimd.partition_broadcast(onyb[:], offs[0:1, 8:9])
                sc_sg = pp.tile([128, 16], F32)
                nc.vector.tensor_scalar_add(sc_sg[:], sc_s[:], onyb[:])

            # ---------------- phase 2: q forward + unbind + inverse --------
            with (
                tc.tile_pool(name="wqp", bufs=1) as wqp,
                tc.tile_pool(name="xin2", bufs=2) as xin2,
                tc.tile_pool(name="shin", bufs=1) as shinp,
                tc.tile_pool(name="zp", bufs=1) as zp,
                tc.tile_pool(name="tmp2", bufs=1) as tp2,
                tc.tile_pool(name="ob", bufs=1) as obp,
                tc.tile_pool(name="ps2", bufs=3, space="PSUM") as ps2,
                tc.tile_pool(name="psi", bufs=3, space="PSUM") as psi,
            ):
                sbxf0 = xin2.tile([128, KT * SBK], F32R, tag="sbx2", bufs=2,
                                  name="sbxf_0")
                for kt in range(KT):
                    nc.sync.dma_start(sbxf0[:, kt * SBK:(kt + 1) * SBK],
                                      xt[kt, :, 0:SBK])
                wq_t = wqp.tile([128, KT * D], F32R, tag="wq")
                for pk in range(KT):
                    nc.sync.dma_start(wq_t[:, pk * D:(pk + 1) * D], wq[pk, :, :])

                for blk in range(NB):
                    s0 = blk * SBK
                    if blk == 0:
                        sbx2 = sbxf0
                    else:
                        sbx2 = xin2.tile([128, KT * SBK], F32R, tag="sbx2",
                                         bufs=2, name=f"sbxf_{blk}")
                        for kt in range(KT):
                            nc.sync.dma_start(sbx2[:, kt * SBK:(kt + 1) * SBK],
                                              xt[kt, :, s0:s0 + SBK])
                    shin = [shinp.tile([128, SBK], F32, tag=f"shi{t}",
                                        name=f"shi{t}_{blk}")
                            for t in range(CT)]
                    for t in range(CT):
                        nc.sync.dma_start(shin[t][:], sp_tiles[t][:, s0:s0 + SBK])
                    zts = [zp.tile([128, SBK], F32R, tag=f"z{t}",
                                    name=f"z{t}_{blk}")
                           for t in range(CT)]

                    # exact nyquist-q for this block -> z row entries
                    nyq = tp2.tile([128, SBK // 128], F32, tag="nyq",
                                   name=f"nyq_{blk}")
                    for j in range(SBK // 128):
                        pnyq = psi.tile([128, 2], F32, tag="nyq", bufs=2)
                        for kt in range(KT):
                            nc.tensor.matmul(
                                pnyq[:],
                                lhsT=sbx2[:, kt * SBK + j * 128:
                                          kt * SBK + (j + 1) * 128].bitcast(F32),
                                rhs=wny_t[:, kt * 4 + 2:kt * 4 + 4],
                                start=(kt == 0), stop=False)
                        nc.tensor.matmul(
                            pnyq[:], lhsT=aux_t[0:1, 0:128].bitcast(F32),
                            rhs=aux_t[0:1, 130:132].bitcast(F32),
                            start=False, stop=True)
                        nc.scalar.activation(nyq[:, j:j + 1], pnyq[:, 0:1], ACT.Copy)
                    nqs = tp2.tile([128, SBK // 128], F32, tag="nqs")
                    nc.vector.tensor_mul(nqs[:], nyq[:], nyq[:])
                    nc.vector.tensor_scalar_add(nqs[:], nqs[:], TINY)
                    nqr = tp2.tile([128, SBK // 128], F32, tag="nqr")
                    nc.vector.reciprocal_approx_fast(out=nqr[:], in_=nqs[:])
                    nc.scalar.activation(nqr[:], nqr[:], ACT.Sqrt)
                    znyb = tp2.tile([128, 16], F32, tag="znyb",
                                    name=f"znyb_{blk}")
                    nc.vector.memset(znyb[:], 0.0)
                    nc.vector.tensor_mul(znyb[:, 0:SBK // 128], nyq[:], nqr[:])
                    nc.vector.tensor_mul(
                        znyb[:, 0:SBK // 128], znyb[:, 0:SBK // 128],
                        sc_sg[:, blk * (SBK // 128):(blk + 1) * (SBK // 128)])
                    pzt = psi.tile([16, 128], F32, tag="nyq", bufs=2,
                                   name=f"pzt_{blk}")
                    nc.tensor.transpose(pzt[:], znyb[:], id_t[:])
                    zt4 = tp2.tile([16, 128], F32R, tag="zt4",
                                   name=f"zt4_{blk}")
                    nc.vector.tensor_copy(zt4[:], pzt[:])
                    for r in range(SBK // 128):
                        row = blk * (SBK // 128) + r
                        nc.sync.dma_start(zflat[0:1, row * 128:(row + 1) * 128],
                                          zt4[r:r + 1, :])

                    for tp in range(4):
                        pq_re = ps2.tile([128, SBK], F32, tag="q")
                        pq_im = ps2.tile([128, SBK], F32, tag="q")
                        for psum_t, ctile in ((pq_re, tp), (pq_im, tp + 4)):
                            for kt in range(KT):
                                nc.tensor.matmul(
                                    psum_t[:],
                                    lhsT=wq_t[:, kt * D + ctile * 128:
                                              kt * D + ctile * 128 + 128],
                                    rhs=sbx2[:, kt * SBK:(kt + 1) * SBK],
                                    start=(kt == 0),
                                    stop=(kt == KT - 1))

                        b_qre = bias_t[:, 16 + tp:17 + tp]
                        b_qim = bias_t[:, 20 + tp:21 + tp]
                        sqa2 = tp2.tile([128, SBK], F32, tag="sqa2")
                        nc.scalar.activation(sqa2[:], pq_re[:], ACT.Square,
                                             bias=b_qre, scale=1.0)
                        sqb2 = tp2.tile([128, SBK], F32, tag="sqb2")
                        nc.scalar.activation(sqb2[:], pq_im[:], ACT.Square,
                                             bias=b_qim, scale=1.0)
                        qq = tp2.tile([128, SBK], F32, tag="qq")
                        nc.vector.scalar_tensor_tensor(
                            out=qq[:], in0=sqa2[:], scalar=TINY, in1=sqb2[:],
                            op0=AO.add, op1=AO.add)
                        r2q = tp2.tile([128, SBK], F32, tag="r2q")
                        nc.vector.reciprocal_approx_fast(out=r2q[:], in_=qq[:])
                        rq = tp2.tile([128, SBK], F32, tag="rq")
                        nc.scalar.activation(rq[:], r2q[:], ACT.Sqrt)
                        qre = tp2.tile([128, SBK], F32, tag="sqa2", name="qre_t")
                        nc.vector.scalar_tensor_tensor(
                            out=qre[:], in0=pq_re[:], scalar=b_qre, in1=rq[:],
                            op0=AO.add, op1=AO.mult)
                        qim = tp2.tile([128, SBK], F32, tag="sqb2", name="qim_t")
                        nc.vector.scalar_tensor_tensor(
                            out=qim[:], in0=pq_im[:], scalar=b_qim, in1=rq[:],
                            op0=AO.add, op1=AO.mult)

                        o_re = offs[:, tp:tp + 1]
                        o_im = offs[:, 4 + tp:5 + tp]
                        t5 = tp2.tile([128, SBK], F32, tag="t5")
                        nc.vector.scalar_tensor_tensor(
                            out=t5[:], in0=shin[tp][:], scalar=o_re,
                            in1=qre[:], op0=AO.add, op1=AO.mult)
                        t6 = tp2.tile([128, SBK], F32, tag="t6")
                        nc.vector.scalar_tensor_tensor(
                            out=t6[:], in0=shin[tp + 4][:], scalar=o_im,
                            in1=qim[:], op0=AO.add, op1=AO.mult)
                        nc.vector.tensor_add(zts[tp][:], t5[:], t6[:])
                        t7 = tp2.tile([128, SBK], F32, tag="t5", name="t7_t")
                        nc.vector.scalar_tensor_tensor(
                            out=t7[:], in0=shin[tp + 4][:], scalar=o_im,
                            in1=qre[:], op0=AO.add, op1=AO.mult)
                        t8 = tp2.tile([128, SBK], F32, tag="t6", name="t8_t")
                        nc.vector.scalar_tensor_tensor(
                            out=t8[:], in0=shin[tp][:], scalar=o_re,
                            in1=qim[:], op0=AO.add, op1=AO.mult)
                        nc.vector.tensor_sub(zts[tp + 4][:], t7[:], t8[:])

                    for ss in range(SBK // 128):
                        row = blk * (SBK // 128) + ss
                        for dh in range(2):
                            po = psi.tile([128, 512], F32, tag="inv")
                            for ct in range(CT):
                                nc.tensor.matmul(
                                    po[:],
                                    lhsT=zts[ct][:, ss * 128:(ss + 1) * 128],
                                    rhs=g_t[:, ct * D + dh * 512:
                                            ct * D + dh * 512 + 512],
                                    start=(ct == 0), stop=False)
                            nc.tensor.matmul(
                                po[:],
                                lhsT=zflat[0:1, row * 128:(row + 1) * 128],
                                rhs=aux_t[0:1, 132 + dh * 512:132 + dh * 512 + 512],
                                start=False, stop=True)
                            ob = obp.tile([128, 512], F32, tag="ob", bufs=3)
                            nc.scalar.activation(ob[:], po[:], ACT.Copy)
                            nc.sync.dma_start(
                                outt[s0 + ss * 128:s0 + (ss + 1) * 128,
                                     dh * 512:(dh + 1) * 512], ob[:])

    nc.compile()
    return nc


_NC_CACHE = None


def _prep_in_maps(x, Wq, bq, Wk, bk, Wv, bv):
    x = np.asarray(x, np.float32)
    Wtq, wnyq = _pack_weight(Wq)
    btq, bnyq = _pack_bias(bq)
    Wtk, wnyk = _pack_weight(Wk)
    btk, bnyk = _pack_bias(bk)
    Wtv, wnyv = _pack_weight(Wv)
    btv, bnyv = _pack_bias(bv)
    G, gny = _build_G()

    def ktile(W):  # [D, D] -> [KT, 128, D]
        return _f32r_round(np.ascontiguousarray(W.reshape(KT, 128, D)))

    wkv_h = np.concatenate([ktile(Wtk), ktile(Wtv)])    # [2*KT, 128, D]
    wq_h = ktile(Wtq)
    g_h = _f32r_round(np.ascontiguousarray(G.reshape(CT, 128, D)))
    wny_kpc = np.stack(
        [wnyk, wnyv, wnyq, np.zeros_like(wnyk)], axis=1).reshape(KT, 128, 4)
    wny_h = np.ascontiguousarray(
        wny_kpc.transpose(1, 0, 2).reshape(128, KT * 4))
    aux_h = np.zeros((1, 1156), np.float32)
    aux_h[0, 0:128] = 1.0
    aux_h[0, 128:131] = [bnyk, bnyv, bnyq]
    aux_h[0, 132:1156] = gny
    aux_h = _f32r_round(aux_h)
    u_h = np.triu(np.ones((128, 128), np.float32))
    id_h = np.eye(128, dtype=np.float32)
    bias_h = np.zeros((128, 24), np.float32)
    for pi, bt in enumerate((btk, btv, btq)):
        bias_h[:, pi * 8:(pi + 1) * 8] = bt.reshape(8, 128).T

    xr = _f32r_round(x)
    in_maps = []
    for c in range(NCORES):
        b, h = c // 2, c % 2
        xs = np.ascontiguousarray(
            xr[b, h * SC:(h + 1) * SC, :].T.reshape(KT, 128, SC))
        m = np.zeros((128, 2), np.float32)
        m[:, 0] = 1.0 if h == 0 else 0.0
        m[:, 1] = 0.0 if h == 0 else 1.0
        in_maps.append(dict(
            xt=xs, wkv=wkv_h, wq=wq_h, g=g_h, wny=wny_h, aux1=aux_h,
            u=u_h, ident=id_h, bias=bias_h, masks=m))
    return in_maps


def kernel(x, Wq, bq, Wk, bk, Wv, bv):
    global _NC_CACHE

    in_maps = _prep_in_maps(x, Wq, bq, Wk, bk, Wv, bv)

    if _NC_CACHE is None:
        _NC_CACHE = _build_nc()
    nc = _NC_CACHE

    trace = bool(int(os.environ.get("KERNEL_TRACE", "0")))
    res = None
    if trace:
        try:
            res = run_bass_kernel_spmd(nc, in_maps, core_ids=list(range(NCORES)),
                                       trace=True)
        except Exception as e:  # ntff hook missing on older axon clients
            print(f"trace unavailable ({e}); rerunning without trace", flush=True)
            res = None
    if res is None:
        res = run_bass_kernel_spmd(nc, in_maps, core_ids=list(range(NCORES)))
    if res.exec_time_ns is not None:
        print(f"HW exec time: {res.exec_time_ns} ns", flush=True)
        kernel.last_exec_time_ns = res.exec_time_ns

    out = np.zeros((B, S, D), np.float32)
    for c in range(NCORES):
        b, h = c // 2, c % 2
        out[b, h * SC:(h + 1) * SC, :] = res.results[c]["out"]
    return out

